# revision 11
# baseline (speedup 1.0000x reference)
"""Trainium2 Bass kernel for nn_DescCGFLoss (retrieval_knn).

Sharding (pure data-parallel, 8 cores): core c handles batch b = c//4 and
anchor rows [(c%4)*256, (c%4)*256+256) — two 128-row tiles of the 1024x1024
per-batch pairwise problem. No cross-core communication.

Per 128-row tile on chip:
  - kd2[i,j] = ||ka_i - kp_j||^2 via a K=5 augmented PE matmul
    ([-2*ka; 1; ||ka||^2] x [kp; ||kp||^2; 1], fp32).
  - q[i,j] = ||p_j||^2 - 2 a_i.p_j via a K=65 augmented PE matmul.
  - three row selections over j (DVE), candidates keep exact values so
    value-match one-hot gathers reproduce the reference argmax choices:
      positive:  argmax (kd2<=2.25) * rand
      far-close: argmax -kd2 - 1000*(kd2<=2.25)   (= argmin kd2 outside)
      outside:   argmax (kd2>2.25) * rand_outside
    then one-hot gather of q + row count of the inside mask.
Host epilogue does the remaining O(B*M) scalar glue (sqrt, weights,
per-batch normalizers) and returns (loss[B,M], active_percentage[B]).
"""

import os
import sys
import numpy as np
from contextlib import ExitStack

for _p in (
    "/root/.axon_site",
    "/root/.axon_site/_ro/trn_rl_repo",
    "/root/.axon_site/_ro/pypackages",
    "/opt/trn_rl_repo",
):
    if os.path.isdir(_p) and _p not in sys.path:
        sys.path.append(_p)

import concourse.bass as bass  # noqa: E402
import concourse.tile as tile  # noqa: E402
from concourse import bacc, mybir  # noqa: E402
from concourse.bass_utils import run_bass_kernel_spmd  # noqa: E402

B, C, M = 2, 64, 1024
P = 128
NCORES = 8
TILES_PER_CORE = 2
ROWS_PER_CORE = P * TILES_PER_CORE  # 256
CGF_RADIUS2 = 2.25  # 1.5**2
TRIPLE_LOSS_GAMMA = 0.5
SIGMA_MAX = 1.0

f32 = mybir.dt.float32


def build_program():
    nc = bacc.Bacc(
        "TRN2", target_bir_lowering=False, debug=False, num_devices=NCORES
    )

    rand_d = nc.dram_tensor("rand", [ROWS_PER_CORE, M], f32, kind="ExternalInput").ap()
    rand_o_d = nc.dram_tensor(
        "rand_o", [ROWS_PER_CORE, M], f32, kind="ExternalInput"
    ).ap()
    ka_aug_d = nc.dram_tensor(
        "ka_aug", [5, ROWS_PER_CORE], f32, kind="ExternalInput"
    ).ap()
    kp_aug_d = nc.dram_tensor("kp_aug", [5, M], f32, kind="ExternalInput").ap()
    a_aug_d = nc.dram_tensor(
        "a_aug", [C + 1, ROWS_PER_CORE], f32, kind="ExternalInput"
    ).ap()
    p_aug_d = nc.dram_tensor("p_aug", [C + 1, M], f32, kind="ExternalInput").ap()
    res_d = nc.dram_tensor(
        "res", [TILES_PER_CORE, P, 4], f32, kind="ExternalOutput"
    ).ap()

    with tile.TileContext(nc) as tc, ExitStack() as ctx:
        const = ctx.enter_context(tc.tile_pool(name="const", bufs=1))
        work = ctx.enter_context(tc.tile_pool(name="work", bufs=2))
        small = ctx.enter_context(tc.tile_pool(name="small", bufs=2))
        psum = ctx.enter_context(tc.tile_pool(name="psum", bufs=2, space="PSUM"))

        Alu = mybir.AluOpType
        Act = mybir.ActivationFunctionType

        # --- prologue: small weight DMAs on the Act HWDGE ring so they don't
        # queue behind the bulk rand loads on the SP ring ---
        ka_aug = const.tile([5, ROWS_PER_CORE], f32, tag="ka_aug")
        nc.scalar.dma_start(ka_aug[:], ka_aug_d[:])
        kp_aug = const.tile([5, M], f32, tag="kp_aug")
        nc.scalar.dma_start(kp_aug[:], kp_aug_d[:])
        a_aug = const.tile([C + 1, ROWS_PER_CORE], f32, tag="a_aug")
        nc.scalar.dma_start(a_aug[:], a_aug_d[:])
        p_aug = const.tile([C + 1, M], f32, tag="p_aug")
        nc.scalar.dma_start(p_aug[:], p_aug_d[:])
        nbias = const.tile([P, 1], f32, tag="nbias")
        nc.vector.memset(nbias[:], -CGF_RADIUS2)

        for t in range(TILES_PER_CORE):
            rand_t = work.tile([P, M], f32, tag="rand")
            nc.sync.dma_start(rand_t[:], rand_d[t * P : (t + 1) * P, :])
            rand_o_t = work.tile([P, M], f32, tag="rand_o")
            nc.sync.dma_start(rand_o_t[:], rand_o_d[t * P : (t + 1) * P, :])

            kd2 = psum.tile([P, M], f32, tag="kd2")
            for j in range(2):
                nc.tensor.matmul(
                    kd2[:, j * 512 : (j + 1) * 512],
                    ka_aug[:, t * P : (t + 1) * P],
                    kp_aug[:, j * 512 : (j + 1) * 512],
                    start=True,
                    stop=True,
                )
            q_ps = psum.tile([P, M], f32, tag="q")
            for j in range(2):
                nc.tensor.matmul(
                    q_ps[:, j * 512 : (j + 1) * 512],
                    a_aug[:, t * P : (t + 1) * P],
                    p_aug[:, j * 512 : (j + 1) * 512],
                    start=True,
                    stop=True,
                )

            res_t = small.tile([P, 4], f32, tag="res")

            # m2 = 1000 * (kd2 < 2.25) on the otherwise-idle ACT engine
            # (sign -> relu(-1000*x) with sum-accum = 1000*n_inside)
            s1 = work.tile([P, M], f32, tag="s1")
            nc.scalar.activation(s1[:], kd2[:], Act.Sign, bias=nbias[:], scale=1.0)
            m2 = work.tile([P, M], f32, tag="m2")
            nc.scalar.activation(
                m2[:], s1[:], Act.Relu, bias=0.0, scale=-1000.0,
                accum_out=res_t[:, 3:4],
            )

            # candidate-value tensors (candidates keep exact values)
            mrand = work.tile([P, M], f32, tag="mrand")
            nc.vector.scalar_tensor_tensor(
                mrand[:], kd2[:], CGF_RADIUS2, rand_t[:], Alu.is_le, Alu.mult
            )
            rmax_p = small.tile([P, 1], f32, tag="rmax_p")
            nc.vector.tensor_reduce(
                rmax_p[:], mrand[:], mybir.AxisListType.X, Alu.max
            )
            mfc = work.tile([P, M], f32, tag="mfc")
            nc.vector.scalar_tensor_tensor(
                mfc[:], m2[:], -1.0, kd2[:], Alu.mult, Alu.subtract
            )
            rmax_f = small.tile([P, 1], f32, tag="rmax_f")
            nc.vector.tensor_reduce(
                rmax_f[:], mfc[:], mybir.AxisListType.X, Alu.max
            )
            orand = work.tile([P, M], f32, tag="orand")
            nc.vector.scalar_tensor_tensor(
                orand[:], kd2[:], CGF_RADIUS2, rand_o_t[:], Alu.is_gt, Alu.mult
            )
            rmax_o = small.tile([P, 1], f32, tag="rmax_o")
            nc.vector.tensor_reduce(
                rmax_o[:], orand[:], mybir.AxisListType.X, Alu.max
            )

            for k, (v, rmax) in enumerate(
                ((mrand, rmax_p), (mfc, rmax_f), (orand, rmax_o))
            ):
                scratch = work.tile([P, M], f32, tag=f"g{k}")
                nc.vector.scalar_tensor_tensor(
                    scratch[:], v[:], rmax[:], q_ps[:],
                    Alu.is_equal, Alu.mult, accum_out=res_t[:, k : k + 1],
                )

            nc.scalar.dma_start(res_d[t], res_t[:])

    nc.compile()
    return nc


_NC = None


def _get_program():
    global _NC
    if _NC is None:
        _NC = build_program()
    return _NC


def shard_inputs(inputs):
    """Build the 8 per-core input maps (with host-augmented matmul weights)."""
    anc_kp = np.asarray(inputs["anc_keypoints"], dtype=np.float32)
    anc_de = np.asarray(inputs["anc_descriptors"], dtype=np.float32)
    pos_kp = np.asarray(inputs["pos_keypoints"], dtype=np.float32)
    pos_de = np.asarray(inputs["pos_descriptors"], dtype=np.float32)
    rand = np.asarray(inputs["random_mat"], dtype=np.float32)
    rand_o = np.asarray(inputs["random_mat_outside"], dtype=np.float32)

    ones_m = np.ones((1, M), np.float32)
    kp_aug = {}
    p_aug = {}
    for b in range(B):
        kpn = (pos_kp[b] ** 2).sum(axis=0, keepdims=True).astype(np.float32)
        kp_aug[b] = np.concatenate([pos_kp[b], kpn, ones_m], axis=0)
        pn = (pos_de[b] ** 2).sum(axis=0, keepdims=True).astype(np.float32)
        p_aug[b] = np.concatenate([pos_de[b], pn], axis=0)

    in_maps = []
    for c in range(NCORES):
        b = c // 4
        r0 = (c % 4) * ROWS_PER_CORE
        rows = slice(r0, r0 + ROWS_PER_CORE)
        ka = anc_kp[b][:, rows]
        kan = (ka ** 2).sum(axis=0, keepdims=True).astype(np.float32)
        ones_r = np.ones((1, ROWS_PER_CORE), np.float32)
        ka_aug = np.concatenate([-2.0 * ka, ones_r, kan], axis=0)
        a_aug = np.concatenate([-2.0 * anc_de[b][:, rows], ones_r], axis=0)
        in_maps.append(
            {
                "rand": np.ascontiguousarray(rand[b, rows, :]),
                "rand_o": np.ascontiguousarray(rand_o[b, rows, :]),
                "ka_aug": np.ascontiguousarray(ka_aug),
                "kp_aug": np.ascontiguousarray(kp_aug[b]),
                "a_aug": np.ascontiguousarray(a_aug),
                "p_aug": np.ascontiguousarray(p_aug[b]),
            }
        )
    return in_maps


def epilogue(res_by_core, inputs):
    """Host glue: combine per-core row results into (loss, active_percentage)."""
    anc_de = np.asarray(inputs["anc_descriptors"], dtype=np.float32)
    sigmas = np.asarray(inputs["anc_sigmas"], dtype=np.float32)
    rms = np.asarray(inputs["random_mat_selection"], dtype=np.float32)

    g_pos = np.empty((B, M), np.float32)
    g_fc = np.empty((B, M), np.float32)
    g_out = np.empty((B, M), np.float32)
    n_in = np.empty((B, M), np.float32)
    for c in range(NCORES):
        b = c // 4
        r0 = (c % 4) * ROWS_PER_CORE
        r = res_by_core[c]["res"].reshape(ROWS_PER_CORE, 4)
        g_pos[b, r0 : r0 + ROWS_PER_CORE] = r[:, 0]
        g_fc[b, r0 : r0 + ROWS_PER_CORE] = r[:, 1]
        g_out[b, r0 : r0 + ROWS_PER_CORE] = r[:, 2]
        n_in[b, r0 : r0 + ROWS_PER_CORE] = r[:, 3]

    anorm2 = (anc_de.astype(np.float32) ** 2).sum(axis=1)  # [B, M]
    pos_dist = np.sqrt(np.maximum(anorm2 + g_pos, 0.0), dtype=np.float32)
    fc_dist = np.sqrt(np.maximum(anorm2 + g_fc, 0.0), dtype=np.float32)
    or_dist = np.sqrt(np.maximum(anorm2 + g_out, 0.0), dtype=np.float32)

    sel = (rms < 0.5).astype(np.float32)
    neg = sel * fc_dist + (np.float32(1.0) - sel) * or_dist

    rowany = (n_in > 0).astype(np.float32)  # positive_mask_BM
    pos_count = rowany.sum(axis=1, dtype=np.float32)  # [B]
    scaling = (np.float32(M) / (pos_count + np.float32(1.0))).astype(np.float32)

    bcl = ((pos_dist - neg + np.float32(TRIPLE_LOSS_GAMMA)) * rowany).astype(
        np.float32
    )
    active = (
        (bcl > np.float32(1e-5)).astype(np.float32).sum(axis=1)
        / (pos_count + np.float32(1.0))
    ).astype(np.float32)

    w = np.maximum(np.float32(SIGMA_MAX) - sigmas, np.float32(0.0)).astype(
        np.float32
    )
    w = (w / w.mean(axis=1, keepdims=True, dtype=np.float32)).astype(np.float32)

    loss = (w * np.maximum(bcl, np.float32(0.0)) * scaling[:, None]).astype(
        np.float32
    )
    return loss, active


def kernel(**inputs):
    nc = _get_program()
    in_maps = shard_inputs(inputs)
    results = run_bass_kernel_spmd(nc, in_maps, list(range(NCORES))).results
    return epilogue(results, inputs)


# revision 12
# speedup vs baseline: 1.0930x; 1.0930x over previous
"""Trainium2 Bass kernel for nn_DescCGFLoss (retrieval_knn).

Sharding (pure data-parallel, 8 cores): core c handles batch b = c//4 and
anchor rows [(c%4)*256, (c%4)*256+256) — two 128-row tiles of the 1024x1024
per-batch pairwise problem. No cross-core communication.

Per 128-row tile on chip:
  - kd2[i,j] = ||ka_i - kp_j||^2 via a K=5 augmented PE matmul
    ([-2*ka; 1; ||ka||^2] x [kp; ||kp||^2; 1], fp32).
  - q[i,j] = ||p_j||^2 - 2 a_i.p_j via a K=65 augmented PE matmul.
  - three row selections over j (DVE), candidates keep exact values so
    value-match one-hot gathers reproduce the reference argmax choices:
      positive:  argmax (kd2<=2.25) * rand
      far-close: argmax -kd2 - 1000*(kd2<=2.25)   (= argmin kd2 outside)
      outside:   argmax (kd2>2.25) * rand_outside
    then one-hot gather of q + row count of the inside mask.
Host epilogue does the remaining O(B*M) scalar glue (sqrt, weights,
per-batch normalizers) and returns (loss[B,M], active_percentage[B]).
"""

import os
import sys
import numpy as np
from contextlib import ExitStack

for _p in (
    "/root/.axon_site",
    "/root/.axon_site/_ro/trn_rl_repo",
    "/root/.axon_site/_ro/pypackages",
    "/opt/trn_rl_repo",
):
    if os.path.isdir(_p) and _p not in sys.path:
        sys.path.append(_p)

import concourse.bass as bass  # noqa: E402
import concourse.tile as tile  # noqa: E402
from concourse import bacc, mybir  # noqa: E402
from concourse.bass_utils import run_bass_kernel_spmd  # noqa: E402

B, C, M = 2, 64, 1024
P = 128
NCORES = 8
TILES_PER_CORE = 2
ROWS_PER_CORE = P * TILES_PER_CORE  # 256
CGF_RADIUS2 = 2.25  # 1.5**2
TRIPLE_LOSS_GAMMA = 0.5
SIGMA_MAX = 1.0

f32 = mybir.dt.float32


def build_program():
    nc = bacc.Bacc(
        "TRN2", target_bir_lowering=False, debug=False, num_devices=NCORES
    )

    rand_d = nc.dram_tensor("rand", [ROWS_PER_CORE, M], f32, kind="ExternalInput").ap()
    rand_o_d = nc.dram_tensor(
        "rand_o", [ROWS_PER_CORE, M], f32, kind="ExternalInput"
    ).ap()
    ka_aug_d = nc.dram_tensor(
        "ka_aug", [5, ROWS_PER_CORE], f32, kind="ExternalInput"
    ).ap()
    kp_aug_d = nc.dram_tensor("kp_aug", [5, M], f32, kind="ExternalInput").ap()
    a_aug_d = nc.dram_tensor(
        "a_aug", [C + 1, ROWS_PER_CORE], f32, kind="ExternalInput"
    ).ap()
    p_aug_d = nc.dram_tensor("p_aug", [C + 1, M], f32, kind="ExternalInput").ap()
    res_d = nc.dram_tensor(
        "res", [TILES_PER_CORE, P, 4], f32, kind="ExternalOutput"
    ).ap()

    with tile.TileContext(nc) as tc, ExitStack() as ctx:
        const = ctx.enter_context(tc.tile_pool(name="const", bufs=1))
        work = ctx.enter_context(tc.tile_pool(name="work", bufs=2))
        small = ctx.enter_context(tc.tile_pool(name="small", bufs=2))
        psum = ctx.enter_context(tc.tile_pool(name="psum", bufs=2, space="PSUM"))

        Alu = mybir.AluOpType
        Act = mybir.ActivationFunctionType

        # --- prologue: small weight DMAs on the Act HWDGE ring so they don't
        # queue behind the bulk rand loads on the SP ring ---
        ka_aug = const.tile([5, ROWS_PER_CORE], f32, tag="ka_aug")
        nc.sync.dma_start(ka_aug[:], ka_aug_d[:])
        kp_aug = const.tile([5, M], f32, tag="kp_aug")
        nc.sync.dma_start(kp_aug[:], kp_aug_d[:])
        a_aug = const.tile([C + 1, ROWS_PER_CORE], f32, tag="a_aug")
        nc.sync.dma_start(a_aug[:], a_aug_d[:])
        p_aug = const.tile([C + 1, M], f32, tag="p_aug")
        nc.sync.dma_start(p_aug[:], p_aug_d[:])
        nbias = const.tile([P, 1], f32, tag="nbias")
        nc.vector.memset(nbias[:], -CGF_RADIUS2)

        for t in range(TILES_PER_CORE):
            rand_t = work.tile([P, M], f32, tag="rand")
            nc.sync.dma_start(rand_t[:], rand_d[t * P : (t + 1) * P, :])
            rand_o_t = work.tile([P, M], f32, tag="rand_o")
            nc.sync.dma_start(rand_o_t[:], rand_o_d[t * P : (t + 1) * P, :])

            kd2 = psum.tile([P, M], f32, tag="kd2")
            for j in range(2):
                nc.tensor.matmul(
                    kd2[:, j * 512 : (j + 1) * 512],
                    ka_aug[:, t * P : (t + 1) * P],
                    kp_aug[:, j * 512 : (j + 1) * 512],
                    start=True,
                    stop=True,
                )
            q_ps = psum.tile([P, M], f32, tag="q")
            for j in range(2):
                nc.tensor.matmul(
                    q_ps[:, j * 512 : (j + 1) * 512],
                    a_aug[:, t * P : (t + 1) * P],
                    p_aug[:, j * 512 : (j + 1) * 512],
                    start=True,
                    stop=True,
                )

            res_t = small.tile([P, 4], f32, tag="res")

            # m2 = 1000 * (kd2 < 2.25) on the otherwise-idle ACT engine
            # (sign -> relu(-1000*x) with sum-accum = 1000*n_inside)
            s1 = work.tile([P, M], f32, tag="s1")
            nc.scalar.activation(s1[:], kd2[:], Act.Sign, bias=nbias[:], scale=1.0)
            m2 = work.tile([P, M], f32, tag="m2")
            nc.scalar.activation(
                m2[:], s1[:], Act.Relu, bias=0.0, scale=-1000.0,
                accum_out=res_t[:, 3:4],
            )

            # candidate-value tensors (candidates keep exact values)
            mrand = work.tile([P, M], f32, tag="mrand")
            nc.vector.scalar_tensor_tensor(
                mrand[:], kd2[:], CGF_RADIUS2, rand_t[:], Alu.is_le, Alu.mult
            )
            rmax_p = small.tile([P, 1], f32, tag="rmax_p")
            nc.vector.tensor_reduce(
                rmax_p[:], mrand[:], mybir.AxisListType.X, Alu.max
            )
            mfc = work.tile([P, M], f32, tag="mfc")
            nc.vector.scalar_tensor_tensor(
                mfc[:], m2[:], -1.0, kd2[:], Alu.mult, Alu.subtract
            )
            rmax_f = small.tile([P, 1], f32, tag="rmax_f")
            nc.vector.tensor_reduce(
                rmax_f[:], mfc[:], mybir.AxisListType.X, Alu.max
            )
            orand = work.tile([P, M], f32, tag="orand")
            nc.vector.scalar_tensor_tensor(
                orand[:], kd2[:], CGF_RADIUS2, rand_o_t[:], Alu.is_gt, Alu.mult
            )
            rmax_o = small.tile([P, 1], f32, tag="rmax_o")
            nc.vector.tensor_reduce(
                rmax_o[:], orand[:], mybir.AxisListType.X, Alu.max
            )

            for k, (v, rmax) in enumerate(
                ((mrand, rmax_p), (mfc, rmax_f), (orand, rmax_o))
            ):
                scratch = work.tile([P, M], f32, tag=f"g{k}")
                nc.vector.scalar_tensor_tensor(
                    scratch[:], v[:], rmax[:], q_ps[:],
                    Alu.is_equal, Alu.mult, accum_out=res_t[:, k : k + 1],
                )

            nc.scalar.dma_start(res_d[t], res_t[:])

    nc.compile()
    return nc


_NC = None


def _get_program():
    global _NC
    if _NC is None:
        _NC = build_program()
    return _NC


def shard_inputs(inputs):
    """Build the 8 per-core input maps (with host-augmented matmul weights)."""
    anc_kp = np.asarray(inputs["anc_keypoints"], dtype=np.float32)
    anc_de = np.asarray(inputs["anc_descriptors"], dtype=np.float32)
    pos_kp = np.asarray(inputs["pos_keypoints"], dtype=np.float32)
    pos_de = np.asarray(inputs["pos_descriptors"], dtype=np.float32)
    rand = np.asarray(inputs["random_mat"], dtype=np.float32)
    rand_o = np.asarray(inputs["random_mat_outside"], dtype=np.float32)

    ones_m = np.ones((1, M), np.float32)
    kp_aug = {}
    p_aug = {}
    for b in range(B):
        kpn = (pos_kp[b] ** 2).sum(axis=0, keepdims=True).astype(np.float32)
        kp_aug[b] = np.concatenate([pos_kp[b], kpn, ones_m], axis=0)
        pn = (pos_de[b] ** 2).sum(axis=0, keepdims=True).astype(np.float32)
        p_aug[b] = np.concatenate([pos_de[b], pn], axis=0)

    in_maps = []
    for c in range(NCORES):
        b = c // 4
        r0 = (c % 4) * ROWS_PER_CORE
        rows = slice(r0, r0 + ROWS_PER_CORE)
        ka = anc_kp[b][:, rows]
        kan = (ka ** 2).sum(axis=0, keepdims=True).astype(np.float32)
        ones_r = np.ones((1, ROWS_PER_CORE), np.float32)
        ka_aug = np.concatenate([-2.0 * ka, ones_r, kan], axis=0)
        a_aug = np.concatenate([-2.0 * anc_de[b][:, rows], ones_r], axis=0)
        in_maps.append(
            {
                "rand": np.ascontiguousarray(rand[b, rows, :]),
                "rand_o": np.ascontiguousarray(rand_o[b, rows, :]),
                "ka_aug": np.ascontiguousarray(ka_aug),
                "kp_aug": np.ascontiguousarray(kp_aug[b]),
                "a_aug": np.ascontiguousarray(a_aug),
                "p_aug": np.ascontiguousarray(p_aug[b]),
            }
        )
    return in_maps


def epilogue(res_by_core, inputs):
    """Host glue: combine per-core row results into (loss, active_percentage)."""
    anc_de = np.asarray(inputs["anc_descriptors"], dtype=np.float32)
    sigmas = np.asarray(inputs["anc_sigmas"], dtype=np.float32)
    rms = np.asarray(inputs["random_mat_selection"], dtype=np.float32)

    g_pos = np.empty((B, M), np.float32)
    g_fc = np.empty((B, M), np.float32)
    g_out = np.empty((B, M), np.float32)
    n_in = np.empty((B, M), np.float32)
    for c in range(NCORES):
        b = c // 4
        r0 = (c % 4) * ROWS_PER_CORE
        r = res_by_core[c]["res"].reshape(ROWS_PER_CORE, 4)
        g_pos[b, r0 : r0 + ROWS_PER_CORE] = r[:, 0]
        g_fc[b, r0 : r0 + ROWS_PER_CORE] = r[:, 1]
        g_out[b, r0 : r0 + ROWS_PER_CORE] = r[:, 2]
        n_in[b, r0 : r0 + ROWS_PER_CORE] = r[:, 3]

    anorm2 = (anc_de.astype(np.float32) ** 2).sum(axis=1)  # [B, M]
    pos_dist = np.sqrt(np.maximum(anorm2 + g_pos, 0.0), dtype=np.float32)
    fc_dist = np.sqrt(np.maximum(anorm2 + g_fc, 0.0), dtype=np.float32)
    or_dist = np.sqrt(np.maximum(anorm2 + g_out, 0.0), dtype=np.float32)

    sel = (rms < 0.5).astype(np.float32)
    neg = sel * fc_dist + (np.float32(1.0) - sel) * or_dist

    rowany = (n_in > 0).astype(np.float32)  # positive_mask_BM
    pos_count = rowany.sum(axis=1, dtype=np.float32)  # [B]
    scaling = (np.float32(M) / (pos_count + np.float32(1.0))).astype(np.float32)

    bcl = ((pos_dist - neg + np.float32(TRIPLE_LOSS_GAMMA)) * rowany).astype(
        np.float32
    )
    active = (
        (bcl > np.float32(1e-5)).astype(np.float32).sum(axis=1)
        / (pos_count + np.float32(1.0))
    ).astype(np.float32)

    w = np.maximum(np.float32(SIGMA_MAX) - sigmas, np.float32(0.0)).astype(
        np.float32
    )
    w = (w / w.mean(axis=1, keepdims=True, dtype=np.float32)).astype(np.float32)

    loss = (w * np.maximum(bcl, np.float32(0.0)) * scaling[:, None]).astype(
        np.float32
    )
    return loss, active


def kernel(**inputs):
    nc = _get_program()
    in_maps = shard_inputs(inputs)
    results = run_bass_kernel_spmd(nc, in_maps, list(range(NCORES))).results
    return epilogue(results, inputs)
